# revision 20
# baseline (speedup 1.0000x reference)
"""Trainium2 Bass kernel for nn_GRUClassifier (B=64, T=512, E=256, H=512, 2-layer BiGRU + FC).

Strategy (8 cores, SPMD single program, zero control flow):
  - Cores form pairs (0,1),(2,3),(4,5),(6,7); each pair redundantly computes the
    full network. Even core runs the forward direction of both layers, odd core
    the backward direction (its `sentence` is time-reversed on the host, so the
    device program is identical).
  - Per core, per layer: a 512-step GRU chain. Each step's gate pre-activations
    are ONE fused PSUM accumulation: bias (K=2 selector matmul) + h@W_hh^T
    (4 K-chunks) + x_t@W_ih^T, using 2-way column-tiled matmuls (two concurrent
    64-wide stationary tiles) so the two 128-col streams run in parallel.
  - Hidden state h is kept in a "stacked" [128, 256] layout (partitions 0:64 =
    h[:, 0:256], 64:128 = h[:, 256:512]) so all gate DVE/ACT ops are
    full-partition-width. h is re-transposed each step with 2 PE transposes.
  - L0 stores transposed outputs y0T to DRAM in 32-step block strips; pairs
    exchange them with a per-block 2-rank AllReduce(SUM) overlapped with the
    L0 chain. L1 recovers the partner's tiles value-free via P = S - own
    (one bulk DVE subtract per block); all L1-side reads are regular DMAs at
    compile-time addresses - no indirect DMA, no rank-dependent addressing.
  - FC: each core computes its direction's half-product [64,10]; the host sums
    the pair and adds fc_b.
"""

import os
import numpy as np
import ml_dtypes

import concourse.bass as bass
import concourse.mybir as mybir
from concourse import bacc, tile
from concourse.bass_utils import run_bass_kernel_spmd

F32 = mybir.dt.float32
BF16 = mybir.dt.bfloat16
I32 = mybir.dt.int32

B = 64
H = 512
E = 256
V = 50000
NCORES = 8
AG_BLOCK = 32   # timesteps per AllReduce block
GB = int(os.environ.get("GB", "8"))  # step-pairs per x-strip load
XK1 = int(os.environ.get("XK1", "8"))  # L1 input k-chunks (debug/timing only)

bf = ml_dtypes.bfloat16

_BUILD_CACHE = {}


# ----------------------------------------------------------------------------
# host-side weight preparation
# ----------------------------------------------------------------------------

def _col_split_AB(WT):
    """WT: [Din, 1536] (cols = r|z|n each 512). Return (A, B) [Din, 768]:
    A = left halves of each gate's columns, B = right halves."""
    r, z, n = WT[:, 0:512], WT[:, 512:1024], WT[:, 1024:1536]
    A = np.concatenate([r[:, 0:256], z[:, 0:256], n[:, 0:256]], axis=1)
    Bm = np.concatenate([r[:, 256:512], z[:, 256:512], n[:, 256:512]], axis=1)
    return np.ascontiguousarray(A), np.ascontiguousarray(Bm)


def _bias_pair_rz(bih, bhh):
    """[2, 512] rows (left,right) of (bih+bhh) for r and z gates."""
    s = bih + bhh
    row0 = np.concatenate([s[0:256], s[512:768]])
    row1 = np.concatenate([s[256:512], s[768:1024]])
    return np.stack([row0, row1])


def _bias_pair_nx(bih, bhh):
    """[2, 512]: cols 0:256 = hn-side bias (bhh_n), 256:512 = xn-side (bih_n)."""
    row0 = np.concatenate([bhh[1024:1280], bih[1024:1280]])
    row1 = np.concatenate([bhh[1280:1536], bih[1280:1536]])
    return np.stack([row0, row1])


_XT_CACHE = {}


def _make_xT(T, sentence, emb, d):
    """Host-side pre-gather + transpose of the embedded sentence for one
    direction: [2, 128, (T//2)*128] bf16 where column 128*s + 64*half + b
    holds x[b, 2s+half, :] split into two 128-dim chunks (partition axis)."""
    key = (T, d)
    if key not in _XT_CACHE:
        sent = np.asarray(sentence)[:, :T]
        if d == 1:
            sent = sent[:, ::-1]
        x = np.asarray(emb, np.float32)[sent]          # [64, T, 256]
        xT = np.ascontiguousarray(
            x.reshape(64, T // 2, 2, 2, 128).transpose(3, 4, 1, 2, 0)
            .reshape(2, 128, (T // 2) * 128).astype(bf))
        _XT_CACHE[key] = xT
    return _XT_CACHE[key]


def make_core_inputs(core, T, sentence, emb, w_ih_l0, w_hh_l0, b_ih_l0, b_hh_l0,
                     w_ih_l1, w_hh_l1, b_ih_l1, b_hh_l1, fc_w, fc_b):
    d = core % 2  # 0 = forward, 1 = backward
    xT = _make_xT(T, sentence, emb, d)

    wh0A, wh0B = _col_split_AB(np.asarray(w_hh_l0)[d].T)   # [512,768]
    wi0A, wi0B = _col_split_AB(np.asarray(w_ih_l0)[d].T)   # [256,768]
    wh1A, wh1B = _col_split_AB(np.asarray(w_hh_l1)[d].T)   # [512,768]
    WT1 = np.asarray(w_ih_l1)[d].T                         # [1024,1536]
    if d == 1:
        # odd core's own direction (k-chunks 0:4) is the bwd half of y0
        WT1 = np.concatenate([WT1[512:1024], WT1[0:512]], axis=0)
    wi1A, wi1B = _col_split_AB(WT1)                        # [1024,768]

    brz0 = _bias_pair_rz(np.asarray(b_ih_l0)[d], np.asarray(b_hh_l0)[d])
    bnx0 = _bias_pair_nx(np.asarray(b_ih_l0)[d], np.asarray(b_hh_l0)[d])
    brz1 = _bias_pair_rz(np.asarray(b_ih_l1)[d], np.asarray(b_hh_l1)[d])
    bnx1 = _bias_pair_nx(np.asarray(b_ih_l1)[d], np.asarray(b_hh_l1)[d])

    sel2 = np.zeros((2, 128), np.float32)
    sel2[0, 0:64] = 1.0
    sel2[1, 64:128] = 1.0

    fch = np.asarray(fc_w)[:, 512 * d:512 * d + 512].T    # [512, 10]
    fcw = np.ascontiguousarray(fch.reshape(4, 128, 10).transpose(1, 0, 2).reshape(128, 40))

    return {
        "xT": xT,
        "wh0A": wh0A.astype(bf), "wh0B": wh0B.astype(bf),
        "wi0A": wi0A.astype(bf), "wi0B": wi0B.astype(bf),
        "wh1A": wh1A.astype(bf), "wh1B": wh1B.astype(bf),
        "wi1A": wi1A.astype(bf), "wi1B": wi1B.astype(bf),
        "brz0": brz0.astype(bf), "bnx0": bnx0.astype(bf),
        "brz1": brz1.astype(bf), "bnx1": bnx1.astype(bf),
        "sel2": sel2.astype(bf),
        "ident": np.eye(128, dtype=np.float32),
        "fcw": fcw.astype(bf),
    }


# ----------------------------------------------------------------------------
# device program
# ----------------------------------------------------------------------------

def build_program(T):
    NB = T // AG_BLOCK
    SW = AG_BLOCK * 128  # strip width (cols per [128, SW] block strip)
    nc = bacc.Bacc("TRN2", target_bir_lowering=False, debug=False,
                   enable_asserts=False, num_devices=NCORES)

    ein = lambda name, shape, dt: nc.dram_tensor(name, shape, dt, kind="ExternalInput")
    xT_d = ein("xT", [2, 128, (T // 2) * 128], BF16)
    wh0A_d = ein("wh0A", [512, 768], BF16); wh0B_d = ein("wh0B", [512, 768], BF16)
    wi0A_d = ein("wi0A", [256, 768], BF16); wi0B_d = ein("wi0B", [256, 768], BF16)
    wh1A_d = ein("wh1A", [512, 768], BF16); wh1B_d = ein("wh1B", [512, 768], BF16)
    wi1A_d = ein("wi1A", [1024, 768], BF16); wi1B_d = ein("wi1B", [1024, 768], BF16)
    brz0_d = ein("brz0", [2, 512], BF16); bnx0_d = ein("bnx0", [2, 512], BF16)
    brz1_d = ein("brz1", [2, 512], BF16); bnx1_d = ein("bnx1", [2, 512], BF16)
    sel2_d = ein("sel2", [2, 128], BF16)
    ident_d = ein("ident", [128, 128], F32)
    fcw_d = ein("fcw", [128, 40], BF16)

    out_d = nc.dram_tensor("out", [64, 10], F32, kind="ExternalOutput")
    dbg0_d = nc.dram_tensor("dbg0", [128, 256], F32, kind="ExternalOutput")
    dbg1_d = nc.dram_tensor("dbg1", [128, 256], F32, kind="ExternalOutput")

    # block-major transposed L0 outputs: [block, j, 128, 32*128]
    y0T = nc.dram_tensor("y0T", [NB, 2, 128, SW], BF16, kind="Internal")
    y0S = nc.dram_tensor("y0S", [NB, 2, 128, SW], BF16, kind="Internal")

    PAIRS = [[0, 1], [2, 3], [4, 5], [6, 7]]

    with tile.TileContext(nc) as tc:
        import contextlib
        ctx = contextlib.ExitStack()
        with ctx:
            cp = ctx.enter_context(tc.tile_pool(name="const", bufs=1))
            # constants into SBUF
            def load_w(dram, kchunks):
                t = cp.tile([128, kchunks * 768], BF16, tag=dram.name)
                for k in range(kchunks):
                    nc.sync.dma_start(out=t[:, 768 * k:768 * (k + 1)],
                                      in_=dram.ap()[128 * k:128 * (k + 1), :])
                return t
            wh0A = load_w(wh0A_d, 4); wh0B = load_w(wh0B_d, 4)
            wi0A = load_w(wi0A_d, 2); wi0B = load_w(wi0B_d, 2)
            wh1A = load_w(wh1A_d, 4); wh1B = load_w(wh1B_d, 4)
            wi1A = load_w(wi1A_d, 8); wi1B = load_w(wi1B_d, 8)

            def load_small(dram, shape, dt):
                t = cp.tile(list(shape), dt, tag=dram.name)
                nc.sync.dma_start(out=t[:, :], in_=dram.ap()[:, :])
                return t
            brz0 = load_small(brz0_d, (2, 512), BF16)
            bnx0 = load_small(bnx0_d, (2, 512), BF16)
            brz1 = load_small(brz1_d, (2, 512), BF16)
            bnx1 = load_small(bnx1_d, (2, 512), BF16)
            sel2 = load_small(sel2_d, (2, 128), BF16)
            ident = load_small(ident_d, (128, 128), F32)
            fcw = load_small(fcw_d, (128, 40), BF16)

            # pools
            prz_p = ctx.enter_context(tc.tile_pool(name="prz", bufs=3, space="PSUM"))
            pnx_p = ctx.enter_context(tc.tile_pool(name="pnx", bufs=2, space="PSUM"))
            ptr_p = ctx.enter_context(tc.tile_pool(name="ptr", bufs=3, space="PSUM"))
            xs_p = [ctx.enter_context(tc.tile_pool(name=f"xs{j}", bufs=3)) for j in (0, 1)]
            h_p = ctx.enter_context(tc.tile_pool(name="h", bufs=3))
            hT_p = [ctx.enter_context(tc.tile_pool(name=f"hT{j}", bufs=2)) for j in (0, 1)]
            sr_p = ctx.enter_context(tc.tile_pool(name="sr", bufs=2))
            oz_p = ctx.enter_context(tc.tile_pool(name="oz", bufs=2))
            t1_p = ctx.enter_context(tc.tile_pool(name="t1", bufs=2))
            t2_p = ctx.enter_context(tc.tile_pool(name="t2", bufs=2))
            nn_p = ctx.enter_context(tc.tile_pool(name="nn", bufs=2))
            zh_p = ctx.enter_context(tc.tile_pool(name="zh", bufs=2))
            nz_p = ctx.enter_context(tc.tile_pool(name="nz", bufs=2))
            so_p = [ctx.enter_context(tc.tile_pool(name=f"so{j}", bufs=2)) for j in (0, 1)]
            sm_p = [ctx.enter_context(tc.tile_pool(name=f"sm{j}", bufs=2)) for j in (0, 1)]
            sp_p = [ctx.enter_context(tc.tile_pool(name=f"sp{j}", bufs=2)) for j in (0, 1)]
            fc_p = ctx.enter_context(tc.tile_pool(name="fc", bufs=1))

            MM = nc.tensor.matmul

            def lhsT_slice(tiles, k):
                # k-chunk k of a [512,*] stationary held as 2 tiles of
                # [128, 128] (cols 0:64 = chunks 0/1, 64:128 = chunks 2/3)
                j, c = k % 2, (k // 2) * 64
                return tiles[j][:, c:c + 64]

            def prep_psum(layer, xsl, xk):
                """Bias + input-side matmuls into fresh psum tiles for one step.
                xsl(k) -> [128, 64] stationary slice for input k-chunk k."""
                prz = prz_p.tile([128, 512], F32)
                pnx = pnx_p.tile([128, 512], F32)
                brz, bnx = (brz0, bnx0) if layer == 0 else (brz1, bnx1)
                wiA, wiB = (wi0A, wi0B) if layer == 0 else (wi1A, wi1B)
                MM(prz[:, :], sel2[:, :], brz[:, :], start=True, stop=False,
                   skip_group_check=True)
                MM(pnx[:, :], sel2[:, :], bnx[:, :], start=True, stop=False,
                   skip_group_check=True)
                for k in range(xk):
                    lt = xsl(k)
                    c0 = 768 * k
                    last = k == xk - 1
                    MM(prz[0:64, :], lt, wiA[:, c0:c0 + 512], start=False,
                       stop=False, skip_group_check=True)
                    MM(pnx[0:64, 256:512], lt, wiA[:, c0 + 512:c0 + 768],
                       start=False, stop=False, skip_group_check=True)
                    MM(prz[64:128, :], lt, wiB[:, c0:c0 + 512], start=False,
                       stop=False, skip_group_check=True)
                    MM(pnx[64:128, 256:512], lt, wiB[:, c0 + 512:c0 + 768],
                       start=False, stop=last, skip_group_check=True)
                return prz, pnx

            def h_matmuls_rnz(layer, hT, prz, pnx, first):
                if first:
                    return  # h0 == 0: no contribution
                whA, whB = (wh0A, wh0B) if layer == 0 else (wh1A, wh1B)
                # r-region first so sigmoid(r) can start while n/z stream
                for k in range(4):
                    lt = lhsT_slice(hT, k)
                    c0 = 768 * k
                    last = k == 3
                    MM(prz[0:64, 0:256], lt, whA[:, c0:c0 + 256], start=False,
                       stop=last, skip_group_check=True)
                    MM(prz[64:128, 0:256], lt, whB[:, c0:c0 + 256], start=False,
                       stop=last, skip_group_check=True)
                for k in range(4):
                    lt = lhsT_slice(hT, k)
                    c0 = 768 * k
                    last = k == 3
                    MM(pnx[0:64, 0:256], lt, whA[:, c0 + 512:c0 + 768],
                       start=False, stop=last, skip_group_check=True)
                    MM(pnx[64:128, 0:256], lt, whB[:, c0 + 512:c0 + 768],
                       start=False, stop=last, skip_group_check=True)
                for k in range(4):
                    lt = lhsT_slice(hT, k)
                    c0 = 768 * k
                    last = k == 3
                    MM(prz[0:64, 256:512], lt, whA[:, c0 + 256:c0 + 512],
                       start=False, stop=last, skip_group_check=True)
                    MM(prz[64:128, 256:512], lt, whB[:, c0 + 256:c0 + 512],
                       start=False, stop=last, skip_group_check=True)
            def h_matmuls_kouter(layer, hT, prz, pnx, first):
                if first:
                    return
                whA, whB = (wh0A, wh0B) if layer == 0 else (wh1A, wh1B)
                # k-outer: each stationary loaded once per (half, k) for r,n,z
                for k in range(4):
                    lt = lhsT_slice(hT, k)
                    c0 = 768 * k
                    last = k == 3
                    MM(prz[0:64, 0:256], lt, whA[:, c0:c0 + 256], start=False,
                       stop=last, skip_group_check=True)
                    MM(pnx[0:64, 0:256], lt, whA[:, c0 + 512:c0 + 768],
                       start=False, stop=last, skip_group_check=True)
                    MM(prz[0:64, 256:512], lt, whA[:, c0 + 256:c0 + 512],
                       start=False, stop=last, skip_group_check=True)
                    MM(prz[64:128, 0:256], lt, whB[:, c0:c0 + 256], start=False,
                       stop=last, skip_group_check=True)
                    MM(pnx[64:128, 0:256], lt, whB[:, c0 + 512:c0 + 768],
                       start=False, stop=last, skip_group_check=True)
                    MM(prz[64:128, 256:512], lt, whB[:, c0 + 256:c0 + 512],
                       start=False, stop=last, skip_group_check=True)

            h_matmuls = (h_matmuls_kouter if os.environ.get("HORD") == "1"
                         else h_matmuls_rnz)


            def gates(prz, pnx, h_prev):
                """h' = (1-z)*tanh(xn + r*hn) + z*h  with oz = 1-z = sigmoid(-zpre).
                Returns (h_new, halves) where halves are the two 128-col slices."""
                sr = sr_p.tile([128, 256], F32)
                nc.scalar.activation(sr[:, :], prz[:, 0:256],
                                     mybir.ActivationFunctionType.Sigmoid)
                oz = oz_p.tile([128, 256], F32)
                nc.scalar.activation(oz[:, :], prz[:, 256:512],
                                     mybir.ActivationFunctionType.Sigmoid,
                                     scale=-1.0)
                t1 = t1_p.tile([128, 256], F32)
                nc.vector.tensor_tensor(out=t1[:, :], in0=sr[:, :],
                                        in1=pnx[:, 0:256], op=mybir.AluOpType.mult)
                t2 = t2_p.tile([128, 256], F32)
                nc.vector.tensor_tensor(out=t2[:, :], in0=t1[:, :],
                                        in1=pnx[:, 256:512], op=mybir.AluOpType.add)
                nn_t = nn_p.tile([128, 256], F32)
                nc.scalar.activation(nn_t[:, :], t2[:, :],
                                     mybir.ActivationFunctionType.Tanh)
                if h_prev is not None:
                    # off critical path: zhneg = (oz - 1)*h  (== -z*h)
                    zh = zh_p.tile([128, 256], F32)
                    nc.vector.scalar_tensor_tensor(
                        out=zh[:, :], in0=oz[:, :], scalar=1.0, in1=h_prev[:, :],
                        op0=mybir.AluOpType.subtract, op1=mybir.AluOpType.mult)
                h_new = h_p.tile([128, 256], F32)
                if h_prev is None:
                    nc.vector.tensor_tensor(out=h_new[:, :], in0=nn_t[:, :],
                                            in1=oz[:, :], op=mybir.AluOpType.mult)
                else:
                    nz = nz_p.tile([128, 256], F32)
                    nc.vector.tensor_tensor(out=nz[:, :], in0=nn_t[:, :],
                                            in1=oz[:, :], op=mybir.AluOpType.mult)
                    # h' = nz - zhneg, per 128-col half so transpose/copy pipeline
                    nc.vector.tensor_tensor(out=h_new[:, 0:128], in0=nz[:, 0:128],
                                            in1=zh[:, 0:128], op=mybir.AluOpType.subtract)
                    nc.vector.tensor_tensor(out=h_new[:, 128:256], in0=nz[:, 128:256],
                                            in1=zh[:, 128:256], op=mybir.AluOpType.subtract)
                return h_new

            def transpose_h(h_new):
                hT = []
                for j in (0, 1):
                    pt = ptr_p.tile([128, 128], F32)
                    nc.tensor.transpose(pt[:, :], h_new[:, 128 * j:128 * (j + 1)],
                                        ident[:, :])
                    ht = hT_p[j].tile([128, 128], BF16)
                    if j == 0:
                        nc.scalar.copy(out=ht[:, :], in_=pt[:, :])
                    else:
                        nc.vector.tensor_copy(out=ht[:, :], in_=pt[:, :])
                    hT.append(ht)
                return hT

            # ---------------- L0 chain ----------------
            NG = (T // 2 + GB - 1) // GB  # strip groups

            def load_xstrip(g):
                """Load GB step-pairs of pre-transposed x: 2 tiles [128, 128*GB]."""
                xs = []
                for k in (0, 1):
                    t = xs_p[k].tile([128, 128 * GB], BF16)
                    nc.sync.dma_start(
                        out=t[:, :],
                        in_=xT_d.ap()[k][:, 128 * GB * g:128 * GB * (g + 1)])
                    xs.append(t)
                return xs

            def xsl_l0(xs, u, half):
                def xsl(k):
                    c = 128 * u + 64 * half
                    return xs[k][:, c:c + 64]
                return xsl

            h_prev, hT = None, None
            xs_cur = load_xstrip(0)
            xs_next = load_xstrip(1) if NG > 1 else None
            preps = {0: prep_psum(0, xsl_l0(xs_cur, 0, 0), 2)}
            for tau in range(T):
                prz, pnx = preps.pop(tau)
                h_matmuls(0, hT, prz, pnx, first=(tau == 0))
                h_new = gates(prz, pnx, h_prev)
                # prep step tau+1 (runs on PE while gates compute on DVE/ACT)
                if tau + 1 < T:
                    nxt = tau + 1
                    pair = nxt // 2
                    if nxt % 2 == 0 and pair % GB == 0:
                        xs_cur = xs_next
                        g = pair // GB + 1
                        if g < NG:
                            xs_next = load_xstrip(g)
                    preps[nxt] = prep_psum(0, xsl_l0(xs_cur, pair % GB, nxt % 2), 2)
                hT = transpose_h(h_new)
                b, pos = tau // AG_BLOCK, tau % AG_BLOCK
                for j in (0, 1):
                    nc.sync.dma_start(out=y0T.ap()[b, j][:, 128 * pos:128 * (pos + 1)],
                                      in_=hT[j][:, :])
                h_prev = h_new
                if (tau + 1) % AG_BLOCK == 0 and not os.environ.get("NO_AG"):
                    nc.gpsimd.collective_compute(
                        "AllReduce", mybir.AluOpType.add,
                        replica_groups=PAIRS,
                        ins=[y0T.ap()[b].opt()],
                        outs=[y0S.ap()[b].opt()])

            nc.sync.dma_start(out=dbg0_d.ap()[:, :], in_=h_prev[:, :])

            # ---------------- L1 chain ----------------
            # strips per block: own[b] (k-chunks 0:4), P[mb]=S[mb]-own[mb]
            # (k-chunks 4:8, partner tiles at mirrored time)
            def load_strips(b):
                mb = NB - 1 - b
                own = []
                for j in (0, 1):
                    t = so_p[j].tile([128, SW], BF16)
                    nc.sync.dma_start(out=t[:, :], in_=y0T.ap()[b, j])
                    own.append(t)
                P = []
                for j in (0, 1):
                    tm = sm_p[j].tile([128, SW], BF16)
                    nc.sync.dma_start(out=tm[:, :], in_=y0T.ap()[mb, j])
                    ts = sp_p[j].tile([128, SW], BF16)
                    nc.sync.dma_start(out=ts[:, :], in_=y0S.ap()[mb, j])
                    nc.gpsimd.tensor_tensor(out=ts[:, :], in0=ts[:, :],
                                            in1=tm[:, :], op=mybir.AluOpType.subtract)
                    P.append(ts)
                return own, P

            def xsl_l1(own, P, tau):
                co = (tau % AG_BLOCK) * 128
                cp_ = (AG_BLOCK - 1 - (tau % AG_BLOCK)) * 128
                def xsl(k):
                    if k < 4:
                        j, c = k % 2, (k // 2) * 64
                        return own[j][:, co + c:co + c + 64]
                    kk = k - 4
                    j, c = kk % 2, (kk // 2) * 64
                    return P[j][:, cp_ + c:cp_ + c + 64]
                return xsl

            h_prev, hT = None, None
            strips = {0: load_strips(0)}
            own, P = strips[0]
            preps = {0: prep_psum(1, xsl_l1(own, P, 0), XK1)}
            for tau in range(T):
                b = tau // AG_BLOCK
                prz, pnx = preps.pop(tau)
                h_matmuls(1, hT, prz, pnx, first=(tau == 0))
                h_new = gates(prz, pnx, h_prev)
                if tau + 1 < T:
                    nxt = tau + 1
                    nb = nxt // AG_BLOCK
                    if nb not in strips:
                        strips[nb] = load_strips(nb)
                        strips.pop(nb - 2, None)
                    own, P = strips[nb]
                    preps[nxt] = prep_psum(1, xsl_l1(own, P, nxt), XK1)
                    # prefetch next block's strips early (mid-block)
                    if nxt % AG_BLOCK == AG_BLOCK // 2 and nb + 1 < NB:
                        strips[nb + 1] = load_strips(nb + 1)
                hT = transpose_h(h_new)
                h_prev = h_new

            nc.sync.dma_start(out=dbg1_d.ap()[:, :], in_=h_prev[:, :])

            # ---------------- FC ----------------
            pfc = prz_p.tile([64, 10], F32, tag="prz")
            for k in range(4):
                MM(pfc[:, :], lhsT_slice(hT, k), fcw[:, 10 * k:10 * (k + 1)],
                   start=(k == 0), stop=(k == 3), skip_group_check=True)
            fco = fc_p.tile([64, 10], F32)
            nc.vector.tensor_copy(out=fco[:, :], in_=pfc[:, :])
            nc.sync.dma_start(out=out_d.ap()[:, :], in_=fco[:, :])

    nc.compile()
    return nc


# ----------------------------------------------------------------------------
# entry point
# ----------------------------------------------------------------------------

def run(T, inputs, trace=False):
    key = T
    if key not in _BUILD_CACHE:
        _BUILD_CACHE[key] = build_program(T)
    nc = _BUILD_CACHE[key]
    in_maps = [make_core_inputs(c, T, **inputs) for c in range(NCORES)]
    res = run_bass_kernel_spmd(nc, in_maps, core_ids=list(range(NCORES)),
                               trace=trace)
    outs = res.results
    fc_b = np.asarray(inputs["fc_b"], np.float32)
    final = np.asarray(outs[0]["out"], np.float32) + np.asarray(outs[1]["out"], np.float32) + fc_b
    return final, res, outs


def kernel(sentence, emb, w_ih_l0, w_hh_l0, b_ih_l0, b_hh_l0,
           w_ih_l1, w_hh_l1, b_ih_l1, b_hh_l1, fc_w, fc_b):
    inputs = dict(sentence=sentence, emb=emb, w_ih_l0=w_ih_l0, w_hh_l0=w_hh_l0,
                  b_ih_l0=b_ih_l0, b_hh_l0=b_hh_l0, w_ih_l1=w_ih_l1,
                  w_hh_l1=w_hh_l1, b_ih_l1=b_ih_l1, b_hh_l1=b_hh_l1,
                  fc_w=fc_w, fc_b=fc_b)
    final, _, _ = run(np.asarray(sentence).shape[1], inputs)
    return final


# revision 21
# speedup vs baseline: 1.8189x; 1.8189x over previous
"""Trainium2 Bass kernel for nn_GRUClassifier (B=64, T=512, E=256, H=512, 2-layer BiGRU + FC).

Strategy (8 cores, SPMD single program, zero control flow):
  - Cores form pairs (0,1),(2,3),(4,5),(6,7); each pair redundantly computes the
    full network. Even core runs the forward direction of both layers, odd core
    the backward direction (its `sentence` is time-reversed on the host, so the
    device program is identical).
  - Per core, per layer: a 512-step GRU chain. Each step's gate pre-activations
    are ONE fused PSUM accumulation: bias (K=2 selector matmul) + h@W_hh^T
    (4 K-chunks) + x_t@W_ih^T, using 2-way column-tiled matmuls (two concurrent
    64-wide stationary tiles) so the two 128-col streams run in parallel.
  - Hidden state h is kept in a "stacked" [128, 256] layout (partitions 0:64 =
    h[:, 0:256], 64:128 = h[:, 256:512]) so all gate DVE/ACT ops are
    full-partition-width. h is re-transposed each step with 2 PE transposes.
  - L0 stores transposed outputs y0T to DRAM in 32-step block strips; pairs
    exchange them with a per-block 2-rank AllReduce(SUM) overlapped with the
    L0 chain. L1 recovers the partner's tiles value-free via P = S - own
    (one bulk DVE subtract per block); all L1-side reads are regular DMAs at
    compile-time addresses - no indirect DMA, no rank-dependent addressing.
  - FC: each core computes its direction's half-product [64,10]; the host sums
    the pair and adds fc_b.
"""

import os
import numpy as np
import ml_dtypes

import concourse.bass as bass
import concourse.mybir as mybir
from concourse import bacc, tile
from concourse.bass_utils import run_bass_kernel_spmd

F32 = mybir.dt.float32
BF16 = mybir.dt.bfloat16
I32 = mybir.dt.int32

B = 64
H = 512
E = 256
V = 50000
NCORES = 8
AG_BLOCK = 32   # timesteps per AllReduce block
GB = int(os.environ.get("GB", "8"))  # step-pairs per x-strip load
XK1 = int(os.environ.get("XK1", "8"))  # L1 input k-chunks (debug/timing only)

bf = ml_dtypes.bfloat16

_BUILD_CACHE = {}


# ----------------------------------------------------------------------------
# host-side weight preparation
# ----------------------------------------------------------------------------

def _col_split_AB(WT):
    """WT: [Din, 1536] (cols = r|z|n each 512). Return (A, B) [Din, 768]:
    A = left halves of each gate's columns, B = right halves."""
    r, z, n = WT[:, 0:512], WT[:, 512:1024], WT[:, 1024:1536]
    A = np.concatenate([r[:, 0:256], z[:, 0:256], n[:, 0:256]], axis=1)
    Bm = np.concatenate([r[:, 256:512], z[:, 256:512], n[:, 256:512]], axis=1)
    return np.ascontiguousarray(A), np.ascontiguousarray(Bm)


def _bias_pair_rz(bih, bhh):
    """[2, 512] rows (left,right) of (bih+bhh) for r and z gates."""
    s = bih + bhh
    row0 = np.concatenate([s[0:256], s[512:768]])
    row1 = np.concatenate([s[256:512], s[768:1024]])
    return np.stack([row0, row1])


def _bias_pair_nx(bih, bhh):
    """[2, 512]: cols 0:256 = hn-side bias (bhh_n), 256:512 = xn-side (bih_n)."""
    row0 = np.concatenate([bhh[1024:1280], bih[1024:1280]])
    row1 = np.concatenate([bhh[1280:1536], bih[1280:1536]])
    return np.stack([row0, row1])


_XT_CACHE = {}


def _make_xT(T, sentence, emb, d):
    """Host-side pre-gather + transpose of the embedded sentence for one
    direction: [2, 128, (T//2)*128] bf16 where column 128*s + 64*half + b
    holds x[b, 2s+half, :] split into two 128-dim chunks (partition axis)."""
    key = (T, d)
    if key not in _XT_CACHE:
        sent = np.asarray(sentence)[:, :T]
        if d == 1:
            sent = sent[:, ::-1]
        x = np.asarray(emb, np.float32)[sent]          # [64, T, 256]
        xT = np.ascontiguousarray(
            x.reshape(64, T // 2, 2, 2, 128).transpose(3, 4, 1, 2, 0)
            .reshape(2, 128, (T // 2) * 128).astype(bf))
        _XT_CACHE[key] = xT
    return _XT_CACHE[key]


def make_core_inputs(core, T, sentence, emb, w_ih_l0, w_hh_l0, b_ih_l0, b_hh_l0,
                     w_ih_l1, w_hh_l1, b_ih_l1, b_hh_l1, fc_w, fc_b):
    d = core % 2  # 0 = forward, 1 = backward
    xT = _make_xT(T, sentence, emb, d)

    wh0A, wh0B = _col_split_AB(np.asarray(w_hh_l0)[d].T)   # [512,768]
    wi0A, wi0B = _col_split_AB(np.asarray(w_ih_l0)[d].T)   # [256,768]
    wh1A, wh1B = _col_split_AB(np.asarray(w_hh_l1)[d].T)   # [512,768]
    WT1 = np.asarray(w_ih_l1)[d].T                         # [1024,1536]
    if d == 1:
        # odd core's own direction (k-chunks 0:4) is the bwd half of y0
        WT1 = np.concatenate([WT1[512:1024], WT1[0:512]], axis=0)
    wi1A, wi1B = _col_split_AB(WT1)                        # [1024,768]

    brz0 = _bias_pair_rz(np.asarray(b_ih_l0)[d], np.asarray(b_hh_l0)[d])
    bnx0 = _bias_pair_nx(np.asarray(b_ih_l0)[d], np.asarray(b_hh_l0)[d])
    brz1 = _bias_pair_rz(np.asarray(b_ih_l1)[d], np.asarray(b_hh_l1)[d])
    bnx1 = _bias_pair_nx(np.asarray(b_ih_l1)[d], np.asarray(b_hh_l1)[d])

    sel2 = np.zeros((2, 128), np.float32)
    sel2[0, 0:64] = 1.0
    sel2[1, 64:128] = 1.0

    fch = np.asarray(fc_w)[:, 512 * d:512 * d + 512].T    # [512, 10]
    fcw = np.ascontiguousarray(fch.reshape(4, 128, 10).transpose(1, 0, 2).reshape(128, 40))

    return {
        "xT": xT,
        "wh0A": wh0A.astype(bf), "wh0B": wh0B.astype(bf),
        "wi0A": wi0A.astype(bf), "wi0B": wi0B.astype(bf),
        "wh1A": wh1A.astype(bf), "wh1B": wh1B.astype(bf),
        "wi1A": wi1A.astype(bf), "wi1B": wi1B.astype(bf),
        "brz0": brz0.astype(bf), "bnx0": bnx0.astype(bf),
        "brz1": brz1.astype(bf), "bnx1": bnx1.astype(bf),
        "sel2": sel2.astype(bf),
        "ident": np.eye(128, dtype=np.float32),
        "fcw": fcw.astype(bf),
    }


# ----------------------------------------------------------------------------
# device program
# ----------------------------------------------------------------------------

def build_program(T):
    NB = T // AG_BLOCK
    SW = AG_BLOCK * 128  # strip width (cols per [128, SW] block strip)
    nc = bacc.Bacc("TRN2", target_bir_lowering=False, debug=False,
                   enable_asserts=False, num_devices=NCORES)

    ein = lambda name, shape, dt: nc.dram_tensor(name, shape, dt, kind="ExternalInput")
    xT_d = ein("xT", [2, 128, (T // 2) * 128], BF16)
    wh0A_d = ein("wh0A", [512, 768], BF16); wh0B_d = ein("wh0B", [512, 768], BF16)
    wi0A_d = ein("wi0A", [256, 768], BF16); wi0B_d = ein("wi0B", [256, 768], BF16)
    wh1A_d = ein("wh1A", [512, 768], BF16); wh1B_d = ein("wh1B", [512, 768], BF16)
    wi1A_d = ein("wi1A", [1024, 768], BF16); wi1B_d = ein("wi1B", [1024, 768], BF16)
    brz0_d = ein("brz0", [2, 512], BF16); bnx0_d = ein("bnx0", [2, 512], BF16)
    brz1_d = ein("brz1", [2, 512], BF16); bnx1_d = ein("bnx1", [2, 512], BF16)
    sel2_d = ein("sel2", [2, 128], BF16)
    ident_d = ein("ident", [128, 128], F32)
    fcw_d = ein("fcw", [128, 40], BF16)

    out_d = nc.dram_tensor("out", [64, 10], F32, kind="ExternalOutput")
    dbg0_d = nc.dram_tensor("dbg0", [128, 256], F32, kind="ExternalOutput")
    dbg1_d = nc.dram_tensor("dbg1", [128, 256], F32, kind="ExternalOutput")

    # block-major transposed L0 outputs: [block, j, 128, 32*128]
    y0T = nc.dram_tensor("y0T", [NB, 2, 128, SW], BF16, kind="Internal")
    y0S = nc.dram_tensor("y0S", [NB, 2, 128, SW], BF16, kind="Internal")

    PAIRS = [[0, 1], [2, 3], [4, 5], [6, 7]]

    with tile.TileContext(nc) as tc:
        import contextlib
        ctx = contextlib.ExitStack()
        with ctx:
            cp = ctx.enter_context(tc.tile_pool(name="const", bufs=1))
            # constants into SBUF
            def load_w(dram, kchunks):
                t = cp.tile([128, kchunks * 768], BF16, tag=dram.name)
                for k in range(kchunks):
                    nc.sync.dma_start(out=t[:, 768 * k:768 * (k + 1)],
                                      in_=dram.ap()[128 * k:128 * (k + 1), :])
                return t
            wh0A = load_w(wh0A_d, 4); wh0B = load_w(wh0B_d, 4)
            wi0A = load_w(wi0A_d, 2); wi0B = load_w(wi0B_d, 2)
            wh1A = load_w(wh1A_d, 4); wh1B = load_w(wh1B_d, 4)
            wi1A = load_w(wi1A_d, 8); wi1B = load_w(wi1B_d, 8)

            def load_small(dram, shape, dt):
                t = cp.tile(list(shape), dt, tag=dram.name)
                nc.sync.dma_start(out=t[:, :], in_=dram.ap()[:, :])
                return t
            brz0 = load_small(brz0_d, (2, 512), BF16)
            bnx0 = load_small(bnx0_d, (2, 512), BF16)
            brz1 = load_small(brz1_d, (2, 512), BF16)
            bnx1 = load_small(bnx1_d, (2, 512), BF16)
            sel2 = load_small(sel2_d, (2, 128), BF16)
            ident = load_small(ident_d, (128, 128), F32)
            fcw = load_small(fcw_d, (128, 40), BF16)

            # pools
            prz_p = ctx.enter_context(tc.tile_pool(name="prz", bufs=3, space="PSUM"))
            pnx_p = ctx.enter_context(tc.tile_pool(name="pnx", bufs=2, space="PSUM"))
            ptr_p = ctx.enter_context(tc.tile_pool(name="ptr", bufs=3, space="PSUM"))
            xs_p = [ctx.enter_context(tc.tile_pool(name=f"xs{j}", bufs=3)) for j in (0, 1)]
            h_p = ctx.enter_context(tc.tile_pool(name="h", bufs=3))
            hT_p = [ctx.enter_context(tc.tile_pool(name=f"hT{j}", bufs=2)) for j in (0, 1)]
            sr_p = ctx.enter_context(tc.tile_pool(name="sr", bufs=2))
            oz_p = ctx.enter_context(tc.tile_pool(name="oz", bufs=2))
            t1_p = ctx.enter_context(tc.tile_pool(name="t1", bufs=2))
            t2_p = ctx.enter_context(tc.tile_pool(name="t2", bufs=2))
            nn_p = ctx.enter_context(tc.tile_pool(name="nn", bufs=2))
            zh_p = ctx.enter_context(tc.tile_pool(name="zh", bufs=2))
            nz_p = ctx.enter_context(tc.tile_pool(name="nz", bufs=2))
            so_p = [ctx.enter_context(tc.tile_pool(name=f"so{j}", bufs=2)) for j in (0, 1)]
            sm_p = [ctx.enter_context(tc.tile_pool(name=f"sm{j}", bufs=2)) for j in (0, 1)]
            sp_p = [ctx.enter_context(tc.tile_pool(name=f"sp{j}", bufs=2)) for j in (0, 1)]
            fc_p = ctx.enter_context(tc.tile_pool(name="fc", bufs=1))

            MM = nc.tensor.matmul

            def lhsT_slice(tiles, k):
                # k-chunk k of a [512,*] stationary held as 2 tiles of
                # [128, 128] (cols 0:64 = chunks 0/1, 64:128 = chunks 2/3)
                j, c = k % 2, (k // 2) * 64
                return tiles[j][:, c:c + 64]

            def prep_psum(layer, xsl, xk):
                """Bias + input-side matmuls into fresh psum tiles for one step.
                xsl(k) -> [128, 64] stationary slice for input k-chunk k."""
                prz = prz_p.tile([128, 512], F32)
                pnx = pnx_p.tile([128, 512], F32)
                brz, bnx = (brz0, bnx0) if layer == 0 else (brz1, bnx1)
                wiA, wiB = (wi0A, wi0B) if layer == 0 else (wi1A, wi1B)
                MM(prz[:, :], sel2[:, :], brz[:, :], start=True, stop=False,
                   skip_group_check=True)
                MM(pnx[:, :], sel2[:, :], bnx[:, :], start=True, stop=False,
                   skip_group_check=True)
                for k in range(xk):
                    lt = xsl(k)
                    c0 = 768 * k
                    last = k == xk - 1
                    MM(prz[0:64, :], lt, wiA[:, c0:c0 + 512], start=False,
                       stop=False, skip_group_check=True)
                    MM(pnx[0:64, 256:512], lt, wiA[:, c0 + 512:c0 + 768],
                       start=False, stop=False, skip_group_check=True)
                    MM(prz[64:128, :], lt, wiB[:, c0:c0 + 512], start=False,
                       stop=False, skip_group_check=True)
                    MM(pnx[64:128, 256:512], lt, wiB[:, c0 + 512:c0 + 768],
                       start=False, stop=last, skip_group_check=True)
                return prz, pnx

            def h_matmuls_rnz(layer, hT, prz, pnx, first):
                if first:
                    return  # h0 == 0: no contribution
                whA, whB = (wh0A, wh0B) if layer == 0 else (wh1A, wh1B)
                # r-region first so sigmoid(r) can start while n/z stream
                for k in range(4):
                    lt = lhsT_slice(hT, k)
                    c0 = 768 * k
                    last = k == 3
                    MM(prz[0:64, 0:256], lt, whA[:, c0:c0 + 256], start=False,
                       stop=last, skip_group_check=True)
                    MM(prz[64:128, 0:256], lt, whB[:, c0:c0 + 256], start=False,
                       stop=last, skip_group_check=True)
                for k in range(4):
                    lt = lhsT_slice(hT, k)
                    c0 = 768 * k
                    last = k == 3
                    MM(pnx[0:64, 0:256], lt, whA[:, c0 + 512:c0 + 768],
                       start=False, stop=last, skip_group_check=True)
                    MM(pnx[64:128, 0:256], lt, whB[:, c0 + 512:c0 + 768],
                       start=False, stop=last, skip_group_check=True)
                for k in range(4):
                    lt = lhsT_slice(hT, k)
                    c0 = 768 * k
                    last = k == 3
                    MM(prz[0:64, 256:512], lt, whA[:, c0 + 256:c0 + 512],
                       start=False, stop=last, skip_group_check=True)
                    MM(prz[64:128, 256:512], lt, whB[:, c0 + 256:c0 + 512],
                       start=False, stop=last, skip_group_check=True)
            def h_matmuls_kouter(layer, hT, prz, pnx, first):
                if first:
                    return
                whA, whB = (wh0A, wh0B) if layer == 0 else (wh1A, wh1B)
                # k-outer: each stationary loaded once per (half, k) for r,n,z
                for k in range(4):
                    lt = lhsT_slice(hT, k)
                    c0 = 768 * k
                    last = k == 3
                    MM(prz[0:64, 0:256], lt, whA[:, c0:c0 + 256], start=False,
                       stop=last, skip_group_check=True)
                    MM(pnx[0:64, 0:256], lt, whA[:, c0 + 512:c0 + 768],
                       start=False, stop=last, skip_group_check=True)
                    MM(prz[0:64, 256:512], lt, whA[:, c0 + 256:c0 + 512],
                       start=False, stop=last, skip_group_check=True)
                    MM(prz[64:128, 0:256], lt, whB[:, c0:c0 + 256], start=False,
                       stop=last, skip_group_check=True)
                    MM(pnx[64:128, 0:256], lt, whB[:, c0 + 512:c0 + 768],
                       start=False, stop=last, skip_group_check=True)
                    MM(prz[64:128, 256:512], lt, whB[:, c0 + 256:c0 + 512],
                       start=False, stop=last, skip_group_check=True)

            h_matmuls = (h_matmuls_kouter if os.environ.get("HORD") == "1"
                         else h_matmuls_rnz)


            def gates(prz, pnx, h_prev):
                """h' = (1-z)*tanh(xn + r*hn) + z*h  with oz = 1-z = sigmoid(-zpre).
                Returns (h_new, halves) where halves are the two 128-col slices."""
                sr = sr_p.tile([128, 256], F32)
                nc.scalar.activation(sr[:, :], prz[:, 0:256],
                                     mybir.ActivationFunctionType.Sigmoid)
                oz = oz_p.tile([128, 256], F32)
                nc.scalar.activation(oz[:, :], prz[:, 256:512],
                                     mybir.ActivationFunctionType.Sigmoid,
                                     scale=-1.0)
                t1 = t1_p.tile([128, 256], F32)
                nc.vector.tensor_tensor(out=t1[:, :], in0=sr[:, :],
                                        in1=pnx[:, 0:256], op=mybir.AluOpType.mult)
                t2 = t2_p.tile([128, 256], F32)
                nc.vector.tensor_tensor(out=t2[:, :], in0=t1[:, :],
                                        in1=pnx[:, 256:512], op=mybir.AluOpType.add)
                nn_t = nn_p.tile([128, 256], F32)
                nc.scalar.activation(nn_t[:, :], t2[:, :],
                                     mybir.ActivationFunctionType.Tanh)
                if h_prev is not None:
                    # off critical path: zhneg = (oz - 1)*h  (== -z*h)
                    zh = zh_p.tile([128, 256], F32)
                    nc.vector.scalar_tensor_tensor(
                        out=zh[:, :], in0=oz[:, :], scalar=1.0, in1=h_prev[:, :],
                        op0=mybir.AluOpType.subtract, op1=mybir.AluOpType.mult)
                h_new = h_p.tile([128, 256], F32)
                if h_prev is None:
                    nc.vector.tensor_tensor(out=h_new[:, :], in0=nn_t[:, :],
                                            in1=oz[:, :], op=mybir.AluOpType.mult)
                else:
                    nz = nz_p.tile([128, 256], F32)
                    nc.vector.tensor_tensor(out=nz[:, :], in0=nn_t[:, :],
                                            in1=oz[:, :], op=mybir.AluOpType.mult)
                    # h' = nz - zhneg, per 128-col half so transpose/copy pipeline
                    nc.vector.tensor_tensor(out=h_new[:, 0:128], in0=nz[:, 0:128],
                                            in1=zh[:, 0:128], op=mybir.AluOpType.subtract)
                    nc.vector.tensor_tensor(out=h_new[:, 128:256], in0=nz[:, 128:256],
                                            in1=zh[:, 128:256], op=mybir.AluOpType.subtract)
                return h_new

            def transpose_h(h_new):
                hT = []
                for j in (0, 1):
                    pt = ptr_p.tile([128, 128], F32)
                    nc.tensor.transpose(pt[:, :], h_new[:, 128 * j:128 * (j + 1)],
                                        ident[:, :])
                    ht = hT_p[j].tile([128, 128], BF16)
                    if j == 0:
                        nc.scalar.copy(out=ht[:, :], in_=pt[:, :])
                    else:
                        nc.vector.tensor_copy(out=ht[:, :], in_=pt[:, :])
                    hT.append(ht)
                return hT

            # ---------------- L0 chain ----------------
            NG = (T // 2 + GB - 1) // GB  # strip groups

            def load_xstrip(g):
                """Load GB step-pairs of pre-transposed x: 2 tiles [128, 128*GB]."""
                xs = []
                for k in (0, 1):
                    t = xs_p[k].tile([128, 128 * GB], BF16)
                    nc.sync.dma_start(
                        out=t[:, :],
                        in_=xT_d.ap()[k][:, 128 * GB * g:128 * GB * (g + 1)])
                    xs.append(t)
                return xs

            def xsl_l0(xs, u, half):
                def xsl(k):
                    c = 128 * u + 64 * half
                    return xs[k][:, c:c + 64]
                return xsl

            h_prev, hT = None, None
            xs_cur = load_xstrip(0)
            xs_next = load_xstrip(1) if NG > 1 else None
            preps = {0: prep_psum(0, xsl_l0(xs_cur, 0, 0), 2)}
            for tau in range(T):
                prz, pnx = preps.pop(tau)
                h_matmuls(0, hT, prz, pnx, first=(tau == 0))
                h_new = gates(prz, pnx, h_prev)
                # prep step tau+1 (runs on PE while gates compute on DVE/ACT)
                if tau + 1 < T:
                    nxt = tau + 1
                    pair = nxt // 2
                    if nxt % 2 == 0 and pair % GB == 0:
                        xs_cur = xs_next
                        g = pair // GB + 1
                        if g < NG:
                            xs_next = load_xstrip(g)
                    preps[nxt] = prep_psum(0, xsl_l0(xs_cur, pair % GB, nxt % 2), 2)
                hT = transpose_h(h_new)
                b, pos = tau // AG_BLOCK, tau % AG_BLOCK
                for j in (0, 1):
                    nc.sync.dma_start(out=y0T.ap()[b, j][:, 128 * pos:128 * (pos + 1)],
                                      in_=hT[j][:, :])
                h_prev = h_new
                if (tau + 1) % AG_BLOCK == 0 and not os.environ.get("NO_AG"):
                    nc.gpsimd.collective_compute(
                        "AllReduce", mybir.AluOpType.add,
                        replica_groups=PAIRS,
                        ins=[y0T.ap()[b].opt()],
                        outs=[y0S.ap()[b].opt()])

            nc.sync.dma_start(out=dbg0_d.ap()[:, :], in_=h_prev[:, :])
            L0ONLY = os.environ.get("L0ONLY") == "1"

            # ---------------- L1 chain ----------------
            # strips per block: own[b] (k-chunks 0:4), P[mb]=S[mb]-own[mb]
            # (k-chunks 4:8, partner tiles at mirrored time)
            def load_strips(b):
                mb = NB - 1 - b
                own = []
                for j in (0, 1):
                    t = so_p[j].tile([128, SW], BF16)
                    nc.sync.dma_start(out=t[:, :], in_=y0T.ap()[b, j])
                    own.append(t)
                P = []
                for j in (0, 1):
                    tm = sm_p[j].tile([128, SW], BF16)
                    nc.sync.dma_start(out=tm[:, :], in_=y0T.ap()[mb, j])
                    ts = sp_p[j].tile([128, SW], BF16)
                    nc.sync.dma_start(out=ts[:, :], in_=y0S.ap()[mb, j])
                    nc.gpsimd.tensor_tensor(out=ts[:, :], in0=ts[:, :],
                                            in1=tm[:, :], op=mybir.AluOpType.subtract)
                    P.append(ts)
                return own, P

            def xsl_l1(own, P, tau):
                co = (tau % AG_BLOCK) * 128
                cp_ = (AG_BLOCK - 1 - (tau % AG_BLOCK)) * 128
                def xsl(k):
                    if k < 4:
                        j, c = k % 2, (k // 2) * 64
                        return own[j][:, co + c:co + c + 64]
                    kk = k - 4
                    j, c = kk % 2, (kk // 2) * 64
                    return P[j][:, cp_ + c:cp_ + c + 64]
                return xsl

            h_prev, hT = None, None
            TL1 = 1 if L0ONLY else T
            strips = {0: load_strips(0)}
            own, P = strips[0]
            preps = {0: prep_psum(1, xsl_l1(own, P, 0), XK1)}
            for tau in range(TL1):
                b = tau // AG_BLOCK
                prz, pnx = preps.pop(tau)
                h_matmuls(1, hT, prz, pnx, first=(tau == 0))
                h_new = gates(prz, pnx, h_prev)
                if tau + 1 < TL1:
                    nxt = tau + 1
                    nb = nxt // AG_BLOCK
                    if nb not in strips:
                        strips[nb] = load_strips(nb)
                        strips.pop(nb - 2, None)
                    own, P = strips[nb]
                    preps[nxt] = prep_psum(1, xsl_l1(own, P, nxt), XK1)
                    # prefetch next block's strips early (mid-block)
                    if nxt % AG_BLOCK == AG_BLOCK // 2 and nb + 1 < NB:
                        strips[nb + 1] = load_strips(nb + 1)
                hT = transpose_h(h_new)
                h_prev = h_new

            nc.sync.dma_start(out=dbg1_d.ap()[:, :], in_=h_prev[:, :])

            # ---------------- FC ----------------
            pfc = prz_p.tile([64, 10], F32, tag="prz")
            for k in range(4):
                MM(pfc[:, :], lhsT_slice(hT, k), fcw[:, 10 * k:10 * (k + 1)],
                   start=(k == 0), stop=(k == 3), skip_group_check=True)
            fco = fc_p.tile([64, 10], F32)
            nc.vector.tensor_copy(out=fco[:, :], in_=pfc[:, :])
            nc.sync.dma_start(out=out_d.ap()[:, :], in_=fco[:, :])

    nc.compile()
    return nc


# ----------------------------------------------------------------------------
# entry point
# ----------------------------------------------------------------------------

def run(T, inputs, trace=False):
    key = T
    if key not in _BUILD_CACHE:
        _BUILD_CACHE[key] = build_program(T)
    nc = _BUILD_CACHE[key]
    in_maps = [make_core_inputs(c, T, **inputs) for c in range(NCORES)]
    res = run_bass_kernel_spmd(nc, in_maps, core_ids=list(range(NCORES)),
                               trace=trace)
    outs = res.results
    fc_b = np.asarray(inputs["fc_b"], np.float32)
    final = np.asarray(outs[0]["out"], np.float32) + np.asarray(outs[1]["out"], np.float32) + fc_b
    return final, res, outs


def kernel(sentence, emb, w_ih_l0, w_hh_l0, b_ih_l0, b_hh_l0,
           w_ih_l1, w_hh_l1, b_ih_l1, b_hh_l1, fc_w, fc_b):
    inputs = dict(sentence=sentence, emb=emb, w_ih_l0=w_ih_l0, w_hh_l0=w_hh_l0,
                  b_ih_l0=b_ih_l0, b_hh_l0=b_hh_l0, w_ih_l1=w_ih_l1,
                  w_hh_l1=w_hh_l1, b_ih_l1=b_ih_l1, b_hh_l1=b_hh_l1,
                  fc_w=fc_w, fc_b=fc_b)
    final, _, _ = run(np.asarray(sentence).shape[1], inputs)
    return final
